# revision 16
# baseline (speedup 1.0000x reference)
"""CTLSTM cell fused kernel for 8 Trainium2 NeuronCores.

Strategy (data-parallel over batch, transposed weights-stationary GEMM):
  - B=16384 rows sharded 2048/core; weights replicated.
  - Compute the TRANSPOSED gate matrix per core: gatesT[3584, 2048] =
    W2T[K2, 3584].T @ xhT[K2, 2048] with the WEIGHT chunk as the PE
    stationary operand ([128,128] per (gate-chunk, k)) and xh as the
    moving operand (N=512 batch columns). Each stationary load serves a
    full-width moving stream, so LDWEIGHTS (97ns) hides entirely under
    the 216ns matmul stream and the PE runs at the bf16 roofline
    (~193us/core for 7.5 GMAC). Outputs stay transposed [H, batch]
    through SBUF and HBM; the host re-transposes and upcasts.
  - Host pre-stages everything in the exact consumption order:
      w[28][128, 8, 128]   bf16 gate-chunk-major (g' = j*7 + t with
        j = H/128 chunk, t in order [d, z, i, f, ib, fb, o]: d first so
        the Ln table switch hides mid-block, o last so the final drain
        feeds only the single h-mul);
      xh[4][128, 8, 512]   bf16 batch-supertile-major;
      ct[4][128, 2048]     bf16 transposed cell state;
      bb[128, 28]          f32 per-partition bias columns (bx+bh,
        negated for the d chunks so sigmoid(-(wd+b)) = ACT(scale=-1,
        bias=-b)).
    DMA triggers cost ~600ns each on the sync queue and can only start
    after the ~7us engine preamble, so the first x/w tiles are k-split
    and ordered so the first matmul starts ~10us in; j=0 iterates
    batch-outer to match arrival order; 8 warmup matmuls on a zeroed
    scratch tile keep the PE busy through the DMA window so the HAM
    clock gate is already 8/8 (2.4GHz) when real matmuls start.
  - PSUM accumulates f32 over 8 k-chunks per (gate-chunk, bsup); the
    ACT engine drains PSUM directly with a FUSED per-partition bias +
    activation into bf16 SBUF stash (no DVE bias pass at all).
  - decay_rate = softplus(wd) = -ln(sigmoid(-wd)) via the ACT Ln table.
  - Elementwise (c, h, c_bar) runs on DVE fully in bf16; all five
    outputs are written bf16 (halves output HBM traffic), host upcasts.
"""

import numpy as np
import ml_dtypes

import concourse.bacc as bacc
import concourse.bass as bass
import concourse.mybir as mybir
import concourse.tile as tile
from concourse.bass_utils import run_bass_kernel_spmd

NCORES = 8
B = 16384
I = 512
H = 512
NG = 7
G = NG * H          # 3584
K2 = I + H          # 1024
P = 128
BS = B // NCORES    # 2048 rows per core
NJ = H // P         # 4 H-chunks
NK = K2 // P        # 8 contraction chunks
NB = 4              # batch supertiles per core
BSUP = BS // NB     # 512
NGC = NG * NJ       # 28 gate chunks

BF16 = mybir.dt.bfloat16
F32 = mybir.dt.float32
AF = mybir.ActivationFunctionType
NPBF16 = ml_dtypes.bfloat16

# per-j gate-type order: d first (early Ln), o LAST so the final drain
# feeds only the single h-mul (shortest tail chain); reference split
# order is i, f, z, o, d, i_bar, f_bar.
T_SRC = [4, 2, 0, 1, 5, 6, 3]   # t -> reference gate index
T_D, T_Z, T_I, T_F, T_IB, T_FB, T_O = range(7)

TRACE = False
LAST_RESULTS = None

_nc_cache = None


def _build():
    nc = bacc.Bacc("TRN2", target_bir_lowering=False, debug=False)

    w_d = nc.dram_tensor("w", [NGC, P, NK, P], BF16, kind="ExternalInput")
    xh_d = nc.dram_tensor("xh", [NB, P, NK, BSUP], BF16, kind="ExternalInput")
    ct_d = nc.dram_tensor("ct", [NJ, P, BS], BF16, kind="ExternalInput")
    bb_d = nc.dram_tensor("bb", [P, NGC], F32, kind="ExternalInput")

    h_d = nc.dram_tensor("h", [NJ, P, BS], BF16, kind="ExternalOutput")
    c_d = nc.dram_tensor("c", [NJ, P, BS], BF16, kind="ExternalOutput")
    cb_d = nc.dram_tensor("cb", [NJ, P, BS], BF16, kind="ExternalOutput")
    o_d = nc.dram_tensor("o", [NJ, P, BS], BF16, kind="ExternalOutput")
    dr_d = nc.dram_tensor("dr", [NJ, P, BS], BF16, kind="ExternalOutput")

    with tile.TileContext(nc) as tc:
        with (
            tc.tile_pool(name="wp", bufs=1) as wp,
            tc.tile_pool(name="xp", bufs=1) as xp,
            tc.tile_pool(name="ctp", bufs=1) as ctp,
            tc.tile_pool(name="bp", bufs=1) as bp,
            tc.tile_pool(name="sp", bufs=2) as sp,
            tc.tile_pool(name="Sp", bufs=2) as Sp,
            tc.tile_pool(name="op", bufs=1) as op,
            tc.tile_pool(name="tp", bufs=2) as tp,
            tc.tile_pool(name="pp", bufs=2, space=bass.MemorySpace.PSUM) as pp,
        ):
            # --- input DMAs, in PE consumption order -------------------
            # j=0 runs batch-outer so the PE queue order matches DMA
            # arrival: bb, w0 (k-halves), x0 (k-halves), w1..w6, x1..x3,
            # ct, then the remaining weight chunks.
            w_sb = [None] * NGC
            xq = [None] * NB
            KH = NK // 2

            def dma_w(gp, split=False):
                w_sb[gp] = wp.tile([P, NK, P], BF16, tag=f"w{gp}", name=f"w{gp}")
                if split:
                    nc.sync.dma_start(w_sb[gp][:, :KH, :], w_d[gp][:, :KH, :])
                    nc.sync.dma_start(w_sb[gp][:, KH:, :], w_d[gp][:, KH:, :])
                else:
                    nc.sync.dma_start(w_sb[gp][:], w_d[gp])

            def dma_x(b, split=False):
                xq[b] = xp.tile([P, NK, BSUP], BF16, tag=f"x{b}", name=f"x{b}")
                if split:
                    nc.sync.dma_start(xq[b][:, :KH, :], xh_d[b][:, :KH, :])
                    nc.sync.dma_start(xq[b][:, KH:, :], xh_d[b][:, KH:, :])
                else:
                    nc.sync.dma_start(xq[b][:], xh_d[b])

            ct_sb = [None] * NJ

            def dma_ct(j):
                ct_sb[j] = ctp.tile([P, BS], BF16, tag=f"ct{j}", name=f"ct{j}")
                nc.sync.dma_start(ct_sb[j][:], ct_d[j])

            # first-MM critical path: x0 in k-quarters interleaved with
            # w0 k-halves so each k-step of the first matmul group lands
            # just-in-time.
            KQ = NK // 4
            xq[0] = xp.tile([P, NK, BSUP], BF16, tag="x0", name="x0")
            w_sb[0] = wp.tile([P, NK, P], BF16, tag="w0", name="w0")
            nc.sync.dma_start(xq[0][:, :KQ, :], xh_d[0][:, :KQ, :])
            nc.sync.dma_start(w_sb[0][:, :KH, :], w_d[0][:, :KH, :])
            nc.sync.dma_start(xq[0][:, KQ:2 * KQ, :], xh_d[0][:, KQ:2 * KQ, :])
            nc.sync.dma_start(w_sb[0][:, KH:, :], w_d[0][:, KH:, :])
            nc.sync.dma_start(xq[0][:, 2 * KQ:3 * KQ, :],
                              xh_d[0][:, 2 * KQ:3 * KQ, :])
            nc.sync.dma_start(xq[0][:, 3 * KQ:, :], xh_d[0][:, 3 * KQ:, :])
            dma_w(1)
            bb = bp.tile([P, NGC], F32, tag="bb")
            nc.sync.dma_start(bb[:], bb_d[:])
            for gp in range(2, NG):
                dma_w(gp)
            # x1 before ct0: j0/b1's matmuls need x1 at ~26us and stall
            # ~0.6us if it queues behind the ct0 transfer; ct0 is only
            # read by j0's DVE elementwise, which has ~10us of slack.
            dma_x(1)
            dma_ct(0)
            for b in range(2, NB):
                dma_x(b)
            for gp in range(NG, NGC):
                dma_w(gp)
            for j in range(1, NJ):
                dma_ct(j)

            # --- PE warmup: keep the array busy while inputs stream in
            # so the HAM clock gate is at 8/8 when real matmuls start.
            scr = tp.tile([P, BSUP], BF16, tag="scr", bufs=1)
            nc.vector.memset(scr[:], 0.0)
            for i in range(6):
                wacc = pp.tile([P, BSUP], F32, tag=f"a{i % 4}", name="wacc")
                nc.tensor.matmul(wacc[:], scr[:, :P], scr[:],
                                 start=True, stop=True)

            # --- main loop --------------------------------------------
            def mm_group(j, t, b, st, S, out_o):
                """8-matmul PSUM accumulation + fused bias/act drain."""
                gp = j * NG + t
                bap = bb[:, gp:gp + 1]
                bsl = slice(b * BSUP, (b + 1) * BSUP)
                acc = pp.tile([P, BSUP], F32, tag=f"a{(t + b) % 4}",
                              name="acc")
                for k in range(NK):
                    nc.tensor.matmul(
                        acc[:], w_sb[gp][:, k, :], xq[b][:, k, :],
                        start=(k == 0), stop=(k == NK - 1),
                    )
                if t == T_D:
                    # S = sigmoid(-(wd + b)) ; bb col holds -b
                    nc.scalar.activation(S[:, bsl], acc[:], AF.Sigmoid,
                                         bias=bap, scale=-1.0)
                elif t == T_Z:
                    nc.scalar.activation(st[T_Z][:, bsl], acc[:], AF.Tanh,
                                         bias=bap)
                elif t == T_O:
                    nc.scalar.activation(out_o[:, bsl], acc[:], AF.Sigmoid,
                                         bias=bap)
                    nc.sync.dma_start(o_d[j][:, bsl], out_o[:, bsl])
                else:
                    nc.scalar.activation(st[t][:, bsl], acc[:], AF.Sigmoid,
                                         bias=bap)

            def elementwise(j, b, st, out_h, out_c, out_cb, out_o):
                """c/tanh first (inputs drain earliest), cb next, h last
                (waits only on the final o drain)."""
                bsl = slice(b * BSUP, (b + 1) * BSUP)
                ctj = ct_sb[j][:, bsl]
                t1 = tp.tile([P, BSUP], BF16, tag="t1")
                t2 = tp.tile([P, BSUP], BF16, tag="t2")
                t3 = tp.tile([P, BSUP], BF16, tag="t3")
                t4 = tp.tile([P, BSUP], BF16, tag="t4")
                nc.vector.tensor_mul(t1[:], st[T_F][:, bsl], ctj)
                nc.vector.tensor_mul(t2[:], st[T_I][:, bsl], st[T_Z][:, bsl])
                nc.vector.tensor_add(out_c[:, bsl], t1[:], t2[:])
                nc.sync.dma_start(c_d[j][:, bsl], out_c[:, bsl])
                nc.scalar.activation(t3[:], out_c[:, bsl], AF.Tanh)
                nc.vector.tensor_mul(t1[:], st[T_FB][:, bsl], ctj)
                nc.vector.tensor_mul(t2[:], st[T_IB][:, bsl], st[T_Z][:, bsl])
                nc.vector.tensor_add(out_cb[:, bsl], t1[:], t2[:])
                nc.sync.dma_start(cb_d[j][:, bsl], out_cb[:, bsl])
                nc.vector.tensor_mul(out_h[:, bsl], out_o[:, bsl], t3[:])
                nc.sync.dma_start(h_d[j][:, bsl], out_h[:, bsl])

            def softplus(j, S, out_dr):
                # dr = -ln(S); one ACT table round-trip per j, hidden
                # under the surrounding matmul stream.
                nc.scalar.activation(S[:], S[:], AF.Ln)
                nc.vector.tensor_scalar_mul(out_dr[:], S[:], -1.0)
                nc.sync.dma_start(dr_d[j], out_dr[:])

            for j in range(NJ):
                st = {}
                for t in (T_Z, T_I, T_F, T_IB, T_FB):
                    st[t] = sp.tile([P, BS], BF16, tag=f"s{t}", name=f"s{t}")
                S = Sp.tile([P, BS], F32, tag="S")
                out_h = op.tile([P, BS], BF16, tag="oh")
                out_c = op.tile([P, BS], BF16, tag="oc")
                out_cb = op.tile([P, BS], BF16, tag="ocb")
                out_o = op.tile([P, BS], BF16, tag="oo")
                out_dr = op.tile([P, BS], BF16, tag="odr")

                if j == 0:
                    # batch-outer: matches the input-DMA arrival order so
                    # the in-order PE queue never waits on a later tile.
                    for b in range(NB):
                        for t in range(NG):
                            mm_group(j, t, b, st, S, out_o)
                        elementwise(j, b, st, out_h, out_c, out_cb, out_o)
                    softplus(j, S, out_dr)
                else:
                    for t in range(NG):
                        gp = j * NG + t
                        for b in range(NB):
                            mm_group(j, t, b, st, S, out_o)
                        if t == T_D:
                            softplus(j, S, out_dr)
                    for b in range(NB):
                        elementwise(j, b, st, out_h, out_c, out_cb, out_o)

    nc.compile()
    return nc


def kernel(x, ht, ct, Wx, bx, Wh, bh):
    global _nc_cache, LAST_RESULTS
    if _nc_cache is None:
        _nc_cache = _build()
    nc = _nc_cache

    x = np.ascontiguousarray(x, dtype=np.float32)
    ht = np.ascontiguousarray(ht, dtype=np.float32)
    ct = np.ascontiguousarray(ct, dtype=np.float32)

    # ---- host staging ------------------------------------------------
    # xhT [K2, B] bf16
    xh_full = np.empty((K2, B), dtype=NPBF16)
    xh_full[:I, :] = x.T.astype(NPBF16)
    xh_full[I:, :] = ht.T.astype(NPBF16)

    WxT = np.asarray(Wx, dtype=np.float32).T   # [512, 3584]
    WhT = np.asarray(Wh, dtype=np.float32).T
    bsum = np.asarray(bx, dtype=np.float32) + np.asarray(bh, dtype=np.float32)

    # wcol [28][128, 8, 128] bf16 in g' = j*7 + t order; bb [128, 28] f32
    w2 = np.empty((K2, G), dtype=np.float32)
    w2[:I, :] = WxT
    w2[I:, :] = WhT
    wcol = np.empty((NGC, P, NK, P), dtype=NPBF16)
    bbp = np.empty((P, NGC), dtype=np.float32)
    for j in range(NJ):
        for t, src in enumerate(T_SRC):
            gp = j * NG + t
            csl = slice(src * H + j * P, src * H + (j + 1) * P)
            blk = w2[:, csl].reshape(NK, P, P)           # [k, p_part, m]
            wcol[gp] = blk.transpose(1, 0, 2).astype(NPBF16)
            sign = -1.0 if t == T_D else 1.0
            bbp[:, gp] = sign * bsum[csl]

    # ctT [512, B] bf16 -> per core [4, 128, 2048]
    ctT = np.ascontiguousarray(ct.T.astype(NPBF16))

    in_maps = []
    for cidx in range(NCORES):
        sl = slice(cidx * BS, (cidx + 1) * BS)
        xc = xh_full[:, sl].reshape(NK, P, NB, BSUP)
        in_maps.append({
            "w": wcol,
            "xh": np.ascontiguousarray(xc.transpose(2, 1, 0, 3)),
            "ct": np.ascontiguousarray(ctT[:, sl]).reshape(NJ, P, BS),
            "bb": bbp,
        })

    res = run_bass_kernel_spmd(nc, in_maps, core_ids=list(range(NCORES)),
                               trace=TRACE)
    LAST_RESULTS = res

    # ---- gather + un-transpose + upcast ------------------------------
    outs = {}
    for name in ("h", "c", "cb", "o", "dr"):
        full = np.empty((B, H), dtype=np.float32)
        for cidx in range(NCORES):
            r = np.asarray(res.results[cidx][name]).reshape(H, BS)
            full[cidx * BS:(cidx + 1) * BS, :] = r.T.astype(np.float32)
        outs[name] = full
    return outs["h"], outs["c"], outs["cb"], outs["o"], outs["dr"]
